# revision 1
# baseline (speedup 1.0000x reference)
"""DistanceLoss kernel for Trainium2 (8 NeuronCores, data-parallel over batch).

Computes mean(MARGIN + dist[i, label_i] - min_{c != label_i} dist[i, c]) where
dist is the pairwise L2 distance between row-normalized WO [N, D] and class
embeddings emb [C, D], via the GEMM identity d2 = x2 + e2 - 2 * WOn @ emb.T.

Per core (2048 rows): PSUM = 2*An@E.T - e2 (fp8e4 DoubleRow matmuls, e2 as an
exact fp16 hi/lo rank-2 matmul), so d2 = 1 - psum. The min over classes !=
label is one custom-DVE TENSOR_MASK_REDUCE per psum tile using an inverted
per-row single-index window (start = col+1 > end = col selects everything
except the label's column) with the two halves chained through the accum
init - exact masked min in a single scan. The label distance goes through a
full-f32 path (indirect-DMA row gather of emb[label], fused multiply-reduce
dot, ScalarE square-accumulate), so matmul quantization never touches it.
rsqrt/sqrt run on DVE via bit-trick seed + Newton steps, keeping ScalarE on a
single LUT table set (Square/Copy) with no table-switch stalls.

Layout tricks: row-block m holds rows {i : i % 16 == m} and class-block c
holds classes {j : j % 16 == c}, which makes every DMA (WO, emb, labels)
contiguous per partition (few large descriptors - descriptor generation, not
bandwidth, dominates DGE cost), at the price of a cheap exact bit-op remap of
the label's matrix column. Loads are split across the SP/Activation HWDGE
queues and issued ahead of any compute on those sequencers; transposes,
GEMMs, and reductions are emission-interleaved so no in-order engine queue
ever convoys behind a late producer.

End-to-end relative error vs the f32 reference: ~3.1e-6 (measured on HW).
Sharding: WO/label split over N across 8 cores, emb replicated; mean on host.
"""

import sys

if "/opt/trn_rl_repo" not in sys.path:
    sys.path.insert(0, "/opt/trn_rl_repo")

import numpy as np

import concourse.bacc as bacc
import concourse.bass as bass
import concourse.mybir as mybir
import concourse.tile as tile
from concourse.bass_utils import run_bass_kernel_spmd
from concourse.dve_ops import TENSOR_MASK_REDUCE, TENSOR_TENSOR_REDUCE
from concourse.masks import make_identity

MARGIN = 1.0
N_CORES = 8
N_FULL, C, D = 16384, 2048, 512
P = 128
NN = N_FULL // N_CORES          # rows per core (2048)
NT = NN // P                    # row tiles per core (16)
CT = C // P                     # class tiles (16)
KT = D // P                     # contraction tiles (4)
HALF = C // 2                   # psum tile width (1024)

f32 = mybir.dt.float32
f16 = mybir.dt.float16
f8 = mybir.dt.float8e4
i32 = mybir.dt.int32
FP8 = True  # fp8e4 DoubleRow main matmuls (measured end-to-end ~1e-6 rel err)
Alu = mybir.AluOpType
Act = mybir.ActivationFunctionType

NEG_BIG = -3.0e38
QUAKE = 0x5F3759DF


def _rsqrt(nc, pool, x_ap, w, name, iters=3):
    """1/sqrt(x) on DVE: bit-trick seed + Newton. x_ap: [P, w] f32."""
    si = pool.tile([P, w], i32, tag=f"rs_i{name}")
    nc.vector.tensor_scalar(
        out=si[:], in0=x_ap.bitcast(i32), scalar1=1, scalar2=0,
        op0=Alu.logical_shift_right, op1=Alu.bitwise_not,
    )
    nc.vector.tensor_scalar(out=si[:], in0=si[:], scalar1=QUAKE + 1, scalar2=None,
                            op0=Alu.add)
    y = pool.tile([P, w], f32, tag=f"rs_y{name}")
    nc.vector.tensor_copy(out=y[:], in_=si[:].bitcast(f32))
    t = pool.tile([P, w], f32, tag=f"rs_t{name}")
    for _ in range(iters):
        nc.vector.tensor_mul(out=t[:], in0=y[:], in1=y[:])
        nc.vector.tensor_mul(out=t[:], in0=t[:], in1=x_ap)
        nc.vector.tensor_scalar(out=t[:], in0=t[:], scalar1=-0.5, scalar2=1.5,
                                op0=Alu.mult, op1=Alu.add)
        nc.vector.tensor_mul(out=y[:], in0=y[:], in1=t[:])
    return y


def _build():
    nc = bacc.Bacc("TRN2", target_bir_lowering=False, debug=False)

    wo_d = nc.dram_tensor("WO", [NN, D], f32, kind="ExternalInput")
    emb_d = nc.dram_tensor("emb", [C, D], f32, kind="ExternalInput")
    lab_d = nc.dram_tensor("label", [NN, 1], i32, kind="ExternalInput")
    out_d = nc.dram_tensor("out", [P, NT], f32, kind="ExternalOutput")

    with tile.TileContext(nc) as tc:
        with (
            tc.tile_pool(name="persist", bufs=1) as pp,
            tc.tile_pool(name="an", bufs=NT) as anp,
            tc.tile_pool(name="ex", bufs=CT) as exp_,
            tc.tile_pool(name="elab", bufs=NT) as elp,
            tc.tile_pool(name="sq", bufs=2) as sqp,
            tc.tile_pool(name="tmp", bufs=8) as tmp_p,
            tc.tile_pool(name="mm", bufs=2, space="PSUM") as mmp,
            tc.tile_pool(name="tp", bufs=4, space="PSUM") as tpp,
        ):
            # ---- constants ----
            ident = pp.tile([P, P], f16)
            make_identity(nc, ident[:])
            identf = pp.tile([P, P], f32)
            make_identity(nc, identf[:])
            ones2 = pp.tile([2, P], f16)
            nc.vector.memset(ones2[:], 1.0)

            # ---- interleaved E + WO pipelines ----
            # Row-block m covers rows {i : i % NT == m} and class-block c covers
            # classes {j : j % CT == c}: partition p's 16 rows are contiguous
            # 32KB in DRAM, so a whole group-of-4 loads as one DMA with 8KB
            # descriptors (descriptor generation, not bandwidth, is the DMA
            # bottleneck).  Matrix column jc = c*128 + p holds class p*16 + c.
            # E tile c: load -> ACT square (e2 col) -> DVE cast (2E fp16)
            # WO tile t: load -> ACT square (x2) -> rnorm (group of 4)
            #            -> DVE cast (An fp16)
            e2c = pp.tile([P, CT], f32)
            x2 = pp.tile([P, NT], f32)
            an = []
            rnorm = pp.tile([P, NT], f32)
            dump = pp.tile([P, 1], f32)
            e2s_dram = nc.dram_tensor("e2scratch", [2, C], f16)
            e2pair = pp.tile([2, C], f16)
            mm_dt = f8 if FP8 else f16
            eT = pp.tile([P, KT, C], mm_dt)
            aT = pp.tile([P, KT, NN], mm_dt)
            e_all = pp.tile([P, CT, D], f32)
            wo_all = pp.tile([P, NT, D], f32)
            emb_v = emb_d.rearrange("(p c) d -> p c d", c=CT)
            wo_v = wo_d.rearrange("(p t) d -> p t d", t=NT)

            # labels first on the Pool queue; group loads up-front so neither
            # HWDGE queue ever waits behind compute issued from the same SEQ
            labi = pp.tile([P, NT], i32)
            nc.gpsimd.dma_start(
                out=labi[:], in_=lab_d[:, 0].rearrange("(p m) -> p m", m=NT))
            for g in range(4):
                sl = slice(g * 4, (g + 1) * 4)
                nc.sync.dma_start(out=e_all[:, sl, :], in_=emb_v[:, sl, :])
                nc.scalar.dma_start(out=wo_all[:, sl, :], in_=wo_v[:, sl, :])

            negmax = pp.tile([P, NT], f32)
            acc0 = pp.tile([P, NT], f32)
            dots = pp.tile([P, NT], f32)
            elab2 = pp.tile([P, NT], f32)
            elab_tiles = [None] * NT
            ex = [None] * CT

            def prep_group(g):
                sl = slice(g * 4, (g + 1) * 4)
                for t in range(g * 4, (g + 1) * 4):
                    s = sqp.tile([P, D], f16, tag="sq", name=f"sq_{t}")
                    nc.scalar.activation(out=s[:], in_=e_all[:, t, :], func=Act.Square,
                                         accum_out=e2c[:, t : t + 1])
                    sw = sqp.tile([P, D], f16, tag="sq", name=f"sqw_{t}")
                    nc.scalar.activation(out=sw[:], in_=wo_all[:, t, :], func=Act.Square,
                                         accum_out=x2[:, t : t + 1])
                # -e2 quarter as fp16 hi/lo rows (DRAM bounce for the
                # partition->free transpose; matrix columns g*512..g*512+511)
                e2n = tmp_p.tile([P, 4], f32, tag="e2n", name=f"e2n_{g}")
                nc.vector.tensor_scalar_mul(out=e2n[:], in0=e2c[:, sl], scalar1=-1.0)
                e2hi = tmp_p.tile([P, 4], f16, tag="e2hi", name=f"e2hi_{g}")
                nc.vector.tensor_copy(out=e2hi[:], in_=e2n[:])
                e2hf = tmp_p.tile([P, 4], f32, tag="e2hf", name=f"e2hf_{g}")
                nc.vector.tensor_copy(out=e2hf[:], in_=e2hi[:])
                e2lo = tmp_p.tile([P, 4], f32, tag="e2lo", name=f"e2lo_{g}")
                nc.vector.tensor_sub(out=e2lo[:], in0=e2n[:], in1=e2hf[:])
                e2lo16 = tmp_p.tile([P, 4], f16, tag="e2lo16", name=f"e2lo16_{g}")
                nc.vector.tensor_copy(out=e2lo16[:], in_=e2lo[:])
                qs = slice(g * 512, (g + 1) * 512)
                nc.sync.dma_start(
                    out=e2s_dram[0:1, qs].rearrange("o (ct p) -> o p ct", p=P),
                    in_=e2hi[:])
                nc.sync.dma_start(
                    out=e2s_dram[1:2, qs].rearrange("o (ct p) -> o p ct", p=P),
                    in_=e2lo16[:])
                nc.sync.dma_start(out=e2pair[:, qs], in_=e2s_dram[:, qs])
                # rnorm + An casts for WO tiles g*4..g*4+3
                y = _rsqrt(nc, tmp_p, x2[:, sl], 4, "n", iters=2)
                nc.vector.tensor_scalar_min(out=rnorm[:, sl], in0=y[:], scalar1=1.0e12)
                for tt in range(g * 4, (g + 1) * 4):
                    a = anp.tile([P, D], f16, tag="an", name=f"an_{tt}")
                    an.append(a)
                    nc.vector.tensor_scalar_mul(out=a[:], in0=wo_all[:, tt, :],
                                                scalar1=rnorm[:, tt : tt + 1])
                # transposes for this group of 4 (E then A)
                for cc in range(g * 4, (g + 1) * 4):
                    # transpose straight from f32 (skips a cast hop); the *2
                    # scale and fp8 cast ride the PSUM->SBUF copy
                    tp = tpp.tile([P, KT, P], f32, tag="tp", name=f"tpe_{cc}")
                    for k in range(KT):
                        nc.tensor.transpose(out=tp[:, k, :],
                                            in_=e_all[:, cc, k * P : (k + 1) * P],
                                            identity=identf[:])
                    if cc % 2 == 0:
                        nc.scalar.activation(out=eT[:, :, cc * P : (cc + 1) * P],
                                             in_=tp[:], func=Act.Copy, scale=2.0)
                    else:
                        nc.vector.tensor_scalar_mul(
                            out=eT[:, :, cc * P : (cc + 1) * P], in0=tp[:],
                            scalar1=2.0)
                for mm in range(g * 4, (g + 1) * 4):
                    tp = tpp.tile([P, KT, P], f16, tag="tp", name=f"tpa_{mm}")
                    for k in range(KT):
                        nc.tensor.transpose(out=tp[:, k, :],
                                            in_=an[mm][:, k * P : (k + 1) * P],
                                            identity=ident[:])
                    if mm % 2 == 1:
                        nc.scalar.copy(out=aT[:, :, mm * P : (mm + 1) * P], in_=tp[:])
                    else:
                        nc.vector.tensor_copy(out=aT[:, :, mm * P : (mm + 1) * P], in_=tp[:])

            pm_tiles = {}

            def mm_mms(h, m):
                pm = mmp.tile([P, HALF], f32, tag="mm", name=f"pm_{h}_{m}")
                pm_tiles[(h, m)] = pm
                for ns in range(2):
                    col0 = h * HALF + ns * 512
                    if FP8:
                        for kp in range(0, KT, 2):
                            nc.tensor.matmul(
                                out=pm[:, ns * 512 : (ns + 1) * 512],
                                lhsT=aT[:, kp : kp + 2, m * P : (m + 1) * P],
                                rhs=eT[:, kp : kp + 2, col0 : col0 + 512],
                                start=(kp == 0), stop=False,
                                perf_mode=mybir.MatmulPerfMode.DoubleRow,
                            )
                    else:
                        for k in range(KT):
                            nc.tensor.matmul(
                                out=pm[:, ns * 512 : (ns + 1) * 512],
                                lhsT=aT[:, k, m * P : (m + 1) * P],
                                rhs=eT[:, k, col0 : col0 + 512],
                                start=(k == 0), stop=False,
                            )
                    nc.tensor.matmul(
                        out=pm[:, ns * 512 : (ns + 1) * 512],
                        lhsT=ones2[:], rhs=e2pair[:, col0 : col0 + 512],
                        start=False, stop=True,
                    )
            def mm_red(h, m):
                st_all = labf1 if h == 0 else labh1
                en_all = labj if h == 0 else labh
                pm = pm_tiles[(h, m)]
                # masked max over c != label (inverted single-index window)
                dmp = tmp_p.tile([P, 1], f32, tag="dmp", name=f"dmp_{h}_{m}")
                nc.vector._custom_dve(
                    TENSOR_MASK_REDUCE,
                    out=dmp[:].broadcast_to([P, HALF]),
                    in0=pm[:],
                    in1=en_all[:, m : m + 1],
                    s0=st_all[:, m : m + 1],
                    s1=NEG_BIG if h == 0 else acc0[:, m : m + 1],
                    imm2=1.0,
                    accum_out=(acc0 if h == 0 else negmax)[:, m : m + 1],
                )
                if h == 0:
                    # label path (f32): gather emb[label] (Pool SEQ descriptor
                    # generation is the gather bottleneck -> start early)
                    g = elp.tile([P, D], f32, tag="elab", name=f"elab_{m}")
                    elab_tiles[m] = g
                    nc.gpsimd.indirect_dma_start(
                        out=g[:], out_offset=None, in_=emb_d[:, :],
                        in_offset=bass.IndirectOffsetOnAxis(
                            ap=labi[:, m : m + 1], axis=0),
                    )
                    s = sqp.tile([P, D], f16, tag="sq", name=f"sql_{m}")
                    nc.scalar.activation(out=s[:], in_=g[:], func=Act.Square,
                                         accum_out=elab2[:, m : m + 1])
                # dots split across both passes to balance DVE per-m load
                if (h == 0 and m % 2 == 0) or (h == 1 and m % 2 == 1):
                    dmp2 = tmp_p.tile([P, 1], f32, tag="dmp", name=f"dmpd_{h}_{m}")
                    nc.vector._custom_dve(
                        TENSOR_TENSOR_REDUCE, out=dmp2[:].broadcast_to([P, D]),
                        in0=wo_all[:, m, :], in1=elab_tiles[m][:], s0=0.0, s1=1.0,
                        accum_out=dots[:, m : m + 1],
                    )

            # pipeline: h0 GEMMs for a group start as soon as eT[0:1024]
            # (groups 0-1) and that group's aT exist; h1 after all transposes
            prep_group(0)
            prep_group(1)
            for m in range(0, 4):
                mm_mms(0, m)
            prep_group(2)
            for m in range(4, 8):
                mm_mms(0, m)
            prep_group(3)
            for m in range(8, 16):
                mm_mms(0, m)
            # label window coordinates: block-major layout [p, m] =
            # label[p*NT + m]; matrix column of class L is (L % CT)*P + L // CT
            # col = (label & 15) << 7 | (label >> 4), in exact int bit ops
            lm = tmp_p.tile([P, NT], i32, tag="lm")
            nc.vector.tensor_scalar(out=lm[:], in0=labi[:], scalar1=15, scalar2=7,
                                    op0=Alu.bitwise_and, op1=Alu.logical_shift_left)
            ld = tmp_p.tile([P, NT], i32, tag="ld")
            nc.vector.tensor_scalar(out=ld[:], in0=labi[:], scalar1=4, scalar2=None,
                                    op0=Alu.logical_shift_right)
            nc.vector.tensor_tensor(out=lm[:], in0=lm[:], in1=ld[:], op=Alu.bitwise_or)
            labj = pp.tile([P, NT], f32)       # column index of label class
            nc.vector.tensor_copy(out=labj[:], in_=lm[:])
            labf1 = pp.tile([P, NT], f32)      # col + 1
            nc.vector.tensor_scalar_add(out=labf1[:], in0=labj[:], scalar1=1.0)
            labh = pp.tile([P, NT], f32)       # col - HALF
            nc.vector.tensor_scalar_add(out=labh[:], in0=labj[:], scalar1=float(-HALF))
            labh1 = pp.tile([P, NT], f32)      # col - HALF + 1
            nc.vector.tensor_scalar_add(out=labh1[:], in0=labj[:], scalar1=float(1 - HALF))

            for m in range(16):
                mm_red(0, m)
            for m in range(16):
                mm_mms(1, m)
                mm_red(1, m)

            # ---- epilogue ----
            # label_d2 = 1 + elab2 - 2*rnorm*dot  (x2 of normalized row == 1)
            ld2 = tmp_p.tile([P, NT], f32, tag="ld2")
            nc.vector.tensor_mul(out=ld2[:], in0=rnorm[:], in1=dots[:])
            nc.vector.tensor_scalar(out=ld2[:], in0=ld2[:], scalar1=-2.0, scalar2=1.0,
                                    op0=Alu.mult, op1=Alu.add)
            nc.vector.tensor_add(out=ld2[:], in0=ld2[:], in1=elab2[:])
            nc.vector.tensor_scalar_max(out=ld2[:], in0=ld2[:], scalar1=0.0)
            # min_{c!=lab} d2 = 1 - negmax
            md2 = tmp_p.tile([P, NT], f32, tag="md2")
            nc.vector.tensor_scalar(out=md2[:], in0=negmax[:], scalar1=-1.0, scalar2=1.0,
                                    op0=Alu.mult, op1=Alu.add)
            nc.vector.tensor_scalar_max(out=md2[:], in0=md2[:], scalar1=0.0)

            # sqrt(x) = x * rsqrt(x); out = sqrt(ld2) - sqrt(md2)
            rl = _rsqrt(nc, tmp_p, ld2[:], NT, "l")
            rm = _rsqrt(nc, tmp_p, md2[:], NT, "m")
            nc.vector.tensor_mul(out=rl[:], in0=rl[:], in1=ld2[:])
            nc.vector.tensor_mul(out=rm[:], in0=rm[:], in1=md2[:])
            outv = pp.tile([P, NT], f32)
            nc.vector.tensor_sub(out=outv[:], in0=rl[:], in1=rm[:])
            nc.gpsimd.dma_start(out=out_d[:, :], in_=outv[:])

    nc.compile()
    return nc


_NC = None


def kernel(WO, emb_weight, label):
    global _NC
    if _NC is None:
        _NC = _build()

    WO = np.ascontiguousarray(np.asarray(WO, dtype=np.float32))
    emb = np.ascontiguousarray(np.asarray(emb_weight, dtype=np.float32))
    lab = np.asarray(label).astype(np.int32).reshape(N_FULL, 1)

    in_maps = []
    for i in range(N_CORES):
        sl = slice(i * NN, (i + 1) * NN)
        in_maps.append({
            "WO": WO[sl],
            "emb": emb,
            "label": np.ascontiguousarray(lab[sl]),
        })
    res = run_bass_kernel_spmd(_NC, in_maps, core_ids=list(range(N_CORES)))
    vals = np.stack([res.results[i]["out"] for i in range(N_CORES)])
    return np.float32(MARGIN + np.mean(vals.astype(np.float64)))



# revision 19
# speedup vs baseline: 3.5371x; 3.5371x over previous
"""DistanceLoss kernel for Trainium2 (8 NeuronCores, data-parallel over batch).

Computes mean(MARGIN + dist[i, label_i] - min_{c != label_i} dist[i, c]) where
dist is the pairwise L2 distance between row-normalized WO [N, D] and class
embeddings emb [C, D], via the GEMM identity d2 = x2 + e2 - 2 * WOn @ emb.T.

Per core (2048 rows): PSUM = 2*An@E.T - e2 (fp8e4 DoubleRow matmuls, e2 as an
exact fp16 hi/lo rank-2 matmul), so d2 = 1 - psum. The min over classes !=
label is one custom-DVE TENSOR_MASK_REDUCE per psum tile using an inverted
per-row single-index window (start = col+1 > end = col selects everything
except the label's column) with the two halves chained through the accum
init - exact masked min in a single scan.

The label distance is computed as |a - e_label|^2 directly: gather emb[label]
rows (cast to f16 during the SWDGE gather), subtract from the normalized f16
row tile on GPSIMD (whose Q7 engine is otherwise idle), and Square-accumulate
on ScalarE. This replaces the old f32 dot-product path (gather + squares +
fused multiply-reduce) at a fraction of the DVE/ScalarE cost.
rsqrt/sqrt run on DVE via bit-trick seed + Newton steps, keeping ScalarE on a
single LUT table set (Square/Copy) with no table-switch stalls.

Layout tricks: row-block m holds rows {i : i % 16 == m} and class-block c
holds classes {j : j % 16 == c}, which makes every DMA (WO, emb, labels)
contiguous per partition (few large descriptors - descriptor generation, not
bandwidth, dominates DGE cost), at the price of a cheap exact bit-op remap of
the label's matrix column. Loads are split across the SP/Activation HWDGE
queues and issued ahead of any compute on those sequencers; transposes,
GEMMs, and reductions are emission-interleaved so no in-order engine queue
ever convoys behind a late producer.

Sharding: WO/label split over N across 8 cores, emb replicated; mean on host.
"""

import sys

if "/opt/trn_rl_repo" not in sys.path:
    sys.path.insert(0, "/opt/trn_rl_repo")

import numpy as np

import concourse.bacc as bacc
import concourse.bass as bass
import concourse.mybir as mybir
import concourse.tile as tile
from concourse.bass_utils import run_bass_kernel_spmd
from concourse.dve_ops import TENSOR_MASK_REDUCE, TENSOR_TENSOR_REDUCE
from concourse.masks import make_identity

MARGIN = 1.0
N_CORES = 8
N_FULL, C, D = 16384, 2048, 512
P = 128
NN = N_FULL // N_CORES          # rows per core (2048)
NT = NN // P                    # row tiles per core (16)
CT = C // P                     # class tiles (16)
KT = D // P                     # contraction tiles (4)
HALF = C // 2                   # psum tile width (1024)

f32 = mybir.dt.float32
f16 = mybir.dt.float16
f8 = mybir.dt.float8e4
i32 = mybir.dt.int32
FP8 = True  # fp8e4 DoubleRow main matmuls (measured end-to-end ~1e-6 rel err)
Alu = mybir.AluOpType
Act = mybir.ActivationFunctionType

NEG_BIG = -3.0e38
QUAKE = 0x5F3759DF


def _rsqrt(nc, pool, x_ap, w, name, iters=3):
    """1/sqrt(x) on DVE: bit-trick seed + Newton. x_ap: [P, w] f32."""
    si = pool.tile([P, w], i32, tag=f"rs_i{name}")
    nc.vector.tensor_scalar(
        out=si[:], in0=x_ap.bitcast(i32), scalar1=1, scalar2=0,
        op0=Alu.logical_shift_right, op1=Alu.bitwise_not,
    )
    nc.vector.tensor_scalar(out=si[:], in0=si[:], scalar1=QUAKE + 1, scalar2=None,
                            op0=Alu.add)
    y = pool.tile([P, w], f32, tag=f"rs_y{name}")
    nc.vector.tensor_copy(out=y[:], in_=si[:].bitcast(f32))
    t = pool.tile([P, w], f32, tag=f"rs_t{name}")
    for _ in range(iters):
        nc.vector.tensor_mul(out=t[:], in0=y[:], in1=y[:])
        nc.vector.tensor_mul(out=t[:], in0=t[:], in1=x_ap)
        nc.vector.tensor_scalar(out=t[:], in0=t[:], scalar1=-0.5, scalar2=1.5,
                                op0=Alu.mult, op1=Alu.add)
        nc.vector.tensor_mul(out=y[:], in0=y[:], in1=t[:])
    return y


def _build():
    nc = bacc.Bacc("TRN2", target_bir_lowering=False, debug=False)

    wo_d = nc.dram_tensor("WO", [NN, D], f32, kind="ExternalInput")
    emb_d = nc.dram_tensor("emb", [C, D], f32, kind="ExternalInput")
    lab_d = nc.dram_tensor("label", [NN, 1], i32, kind="ExternalInput")
    out_d = nc.dram_tensor("out", [P, NT], f32, kind="ExternalOutput")

    with tile.TileContext(nc) as tc:
        with (
            tc.tile_pool(name="persist", bufs=1) as pp,
            tc.tile_pool(name="an", bufs=NT) as anp,
            tc.tile_pool(name="elab", bufs=8) as elp,
            tc.tile_pool(name="dif", bufs=NT) as dfp,
            tc.tile_pool(name="sq", bufs=2) as sqp,
            tc.tile_pool(name="tmp", bufs=8) as tmp_p,
            tc.tile_pool(name="mm", bufs=2, space="PSUM") as mmp,
            tc.tile_pool(name="tpe", bufs=2, space="PSUM") as tpe_p,
            tc.tile_pool(name="tpa", bufs=2, space="PSUM") as tpa_p,
        ):
            # ---- interleaved E + WO pipelines ----
            # Row-block m covers rows {i : i % NT == m} and class-block c covers
            # classes {j : j % CT == c}: partition p's 16 rows are contiguous
            # 32KB in DRAM, so a whole group-of-4 loads as one DMA with 8KB
            # descriptors (descriptor generation, not bandwidth, is the DMA
            # bottleneck).  Matrix column jc = c*128 + p holds class p*16 + c.
            e2c = pp.tile([P, CT], f32)
            x2 = pp.tile([P, NT], f32)
            an = [None] * NT
            rnorm = pp.tile([P, NT], f32)
            e2s_dram = nc.dram_tensor("e2scratch", [2, C], f16)
            e2pair = pp.tile([2, C], f16)
            mm_dt = f8 if FP8 else f16
            eT = pp.tile([P, KT, C], mm_dt)
            aT = pp.tile([P, KT, NN], mm_dt)
            e_all = pp.tile([P, CT, D], f32)
            wo_all = pp.tile([P, NT, D], f32)
            emb_v = emb_d.rearrange("(p c) d -> p c d", c=CT)
            wo_v = wo_d.rearrange("(p t) d -> p t d", t=NT)

            # labels first on the Pool queue (everything label-side hangs off
            # this), then the identity tiles (Pool-engine memset/affine ops
            # needed by the first transposes), then the row gathers.
            labi = pp.tile([P, NT], i32)
            nc.gpsimd.dma_start(
                out=labi[:], in_=lab_d[:, 0].rearrange("(p m) -> p m", m=NT))

            ident = pp.tile([P, P], f16)
            make_identity(nc, ident[:])
            identf = pp.tile([P, P], f32)
            make_identity(nc, identf[:])
            ones2 = pp.tile([2, P], f16)
            nc.vector.memset(ones2[:], 1.0)

            for g in range(4):
                sl = slice(g * 4, (g + 1) * 4)
                nc.sync.dma_start(out=e_all[:, sl, :], in_=emb_v[:, sl, :])
                nc.scalar.dma_start(out=wo_all[:, sl, :], in_=wo_v[:, sl, :])

            # emb[label] row gathers: depend only on labi, issue them all
            # now so the Pool SEQ's descriptor generation runs underneath the
            # load/transpose phase.  The gather casts f32 -> f16 in-flight
            # (SWDGE dma-with-cast), halving wire traffic; f16 embedding rows
            # are plenty for the |a - e|^2 label distance (~1e-5 end-to-end).
            elab_tiles = [None] * NT

            def gather(m):
                g16 = elp.tile([P, D], f16, tag="elab", name=f"elab_{m}")
                elab_tiles[m] = g16
                nc.gpsimd.indirect_dma_start(
                    out=g16[:], out_offset=None, in_=emb_d[:, :],
                    in_offset=bass.IndirectOffsetOnAxis(
                        ap=labi[:, m : m + 1], axis=0),
                )

            # Only the first 8 gathers go out now: the SWDGE descriptor ring
            # holds 1024 descriptors (= 8 row gathers), and their wire slots
            # contend with the big input loads; more here would stall the
            # Pool queue ahead of the aT evictions that gate the GEMMs.  The
            # rest are issued after the last aT eviction.
            for m in range(8):
                gather(m)

            negmax = pp.tile([P, NT], f32)
            acc0 = pp.tile([P, NT], f32)
            ld2 = pp.tile([P, NT], f32)

            # ---- emission helpers (each touches a single engine queue) ----
            def sq_e(t):            # ACT: e2 square for class-tile t
                s = sqp.tile([P, D], f16, tag="sq", name=f"sq_{t}")
                nc.scalar.activation(out=s[:], in_=e_all[:, t, :], func=Act.Square,
                                     accum_out=e2c[:, t : t + 1])

            def ttr_e(t):           # DVE: e2 sum via fused multiply-reduce
                # (groups 0-1 only: fills DVE's otherwise-idle startup window
                # and halves the ACT square queue that gates e2pair -> GEMM)
                dm = tmp_p.tile([P, 1], f32, tag="dmp", name=f"te_{t}")
                nc.vector._custom_dve(
                    TENSOR_TENSOR_REDUCE, out=dm[:].broadcast_to([P, D]),
                    in0=e_all[:, t, :], in1=e_all[:, t, :], s0=0.0, s1=1.0,
                    accum_out=e2c[:, t : t + 1],
                )

            def sq_w(t):            # ACT: x2 square for row-tile t
                sw = sqp.tile([P, D], f16, tag="sq", name=f"sqw_{t}")
                nc.scalar.activation(out=sw[:], in_=wo_all[:, t, :], func=Act.Square,
                                     accum_out=x2[:, t : t + 1])

            def e2bits(g):          # DVE + sync-DMA: -e2 hi/lo bounce
                sl = slice(g * 4, (g + 1) * 4)
                e2n = tmp_p.tile([P, 4], f32, tag="e2n", name=f"e2n_{g}")
                nc.vector.tensor_scalar_mul(out=e2n[:], in0=e2c[:, sl], scalar1=-1.0)
                e2hi = tmp_p.tile([P, 4], f16, tag="e2hi", name=f"e2hi_{g}")
                nc.vector.tensor_copy(out=e2hi[:], in_=e2n[:])
                e2hf = tmp_p.tile([P, 4], f32, tag="e2hf", name=f"e2hf_{g}")
                nc.vector.tensor_copy(out=e2hf[:], in_=e2hi[:])
                e2lo = tmp_p.tile([P, 4], f32, tag="e2lo", name=f"e2lo_{g}")
                nc.vector.tensor_sub(out=e2lo[:], in0=e2n[:], in1=e2hf[:])
                e2lo16 = tmp_p.tile([P, 4], f16, tag="e2lo16", name=f"e2lo16_{g}")
                nc.vector.tensor_copy(out=e2lo16[:], in_=e2lo[:])
                qs = slice(g * 512, (g + 1) * 512)
                nc.sync.dma_start(
                    out=e2s_dram[0:1, qs].rearrange("o (ct p) -> o p ct", p=P),
                    in_=e2hi[:])
                nc.sync.dma_start(
                    out=e2s_dram[1:2, qs].rearrange("o (ct p) -> o p ct", p=P),
                    in_=e2lo16[:])
                nc.sync.dma_start(out=e2pair[:, qs], in_=e2s_dram[:, qs])

            def rnorm_an(g):        # DVE: rsqrt(x2) + An f16 casts
                sl = slice(g * 4, (g + 1) * 4)
                y = _rsqrt(nc, tmp_p, x2[:, sl], 4, "n", iters=2)
                nc.vector.tensor_scalar_min(out=rnorm[:, sl], in0=y[:], scalar1=1.0e12)
                for tt in range(g * 4, (g + 1) * 4):
                    a = anp.tile([P, D], f16, tag="an", name=f"an_{tt}")
                    an[tt] = a
                    nc.vector.tensor_scalar_mul(out=a[:], in0=wo_all[:, tt, :],
                                                scalar1=rnorm[:, tt : tt + 1])

            tpe_tiles = {}
            tpa_tiles = {}

            def tpe(g):             # PE: E transposes for class group g
                for cc in range(g * 4, (g + 1) * 4):
                    tp = tpe_p.tile([P, KT, P], f32, tag="tpe", name=f"tpe_{cc}")
                    tpe_tiles[cc] = tp
                    for k in range(KT):
                        nc.tensor.transpose(out=tp[:, k, :],
                                            in_=e_all[:, cc, k * P : (k + 1) * P],
                                            identity=identf[:])

            def tpa(g):             # PE: A transposes for row group g
                for mm in range(g * 4, (g + 1) * 4):
                    tp = tpa_p.tile([P, KT, P], f16, tag="tpa", name=f"tpa_{mm}")
                    tpa_tiles[mm] = tp
                    for k in range(KT):
                        nc.tensor.transpose(out=tp[:, k, :],
                                            in_=an[mm][:, k * P : (k + 1) * P],
                                            identity=ident[:])

            def ev_e(g):            # ACT: eT psum->sbuf with *2 scale + fp8
                for cc in range(g * 4, (g + 1) * 4):
                    nc.scalar.activation(out=eT[:, :, cc * P : (cc + 1) * P],
                                         in_=tpe_tiles[cc][:], func=Act.Copy,
                                         scale=2.0)

            def ev_a(g):            # aT psum->sbuf fp8, split DVE/ACT; the
                # Pool queue must stay clear of these: it stalls on gather
                # wire semaphores behind the big input loads.
                for mm in range(g * 4, (g + 1) * 4):
                    if mm % 2 == 1:
                        nc.scalar.copy(out=aT[:, :, mm * P : (mm + 1) * P],
                                       in_=tpa_tiles[mm][:])
                    else:
                        nc.vector.tensor_copy(out=aT[:, :, mm * P : (mm + 1) * P],
                                              in_=tpa_tiles[mm][:])
            def label_dist(mm):
                # |a - e_label|^2: GPSIMD does the f16 subtract (its Q7
                # engine is otherwise idle), ScalarE square-accumulates into
                # ld2.  Emitted after all prep groups so these never block
                # the in-order ACT/Pool queues ahead of squares/transposes;
                # they drain during the GEMM+scan phase when ACT idles.
                df = dfp.tile([P, D], f16, tag="dif", name=f"dif_{mm}")
                nc.gpsimd.tensor_tensor(out=df[:], in0=an[mm][:],
                                        in1=elab_tiles[mm][:], op=Alu.subtract)
                sd = sqp.tile([P, D], f16, tag="sq", name=f"sqd_{mm}")
                nc.scalar.activation(out=sd[:], in_=df[:], func=Act.Square,
                                     accum_out=ld2[:, mm : mm + 1])

            pm_tiles = {}

            def mm_mms(h, m):
                pm = mmp.tile([P, HALF], f32, tag="mm", name=f"pm_{h}_{m}")
                pm_tiles[(h, m)] = pm
                for ns in range(2):
                    col0 = h * HALF + ns * 512
                    if FP8:
                        for kp in range(0, KT, 2):
                            nc.tensor.matmul(
                                out=pm[:, ns * 512 : (ns + 1) * 512],
                                lhsT=aT[:, kp : kp + 2, m * P : (m + 1) * P],
                                rhs=eT[:, kp : kp + 2, col0 : col0 + 512],
                                start=(kp == 0), stop=False,
                                perf_mode=mybir.MatmulPerfMode.DoubleRow,
                            )
                    else:
                        for k in range(KT):
                            nc.tensor.matmul(
                                out=pm[:, ns * 512 : (ns + 1) * 512],
                                lhsT=aT[:, k, m * P : (m + 1) * P],
                                rhs=eT[:, k, col0 : col0 + 512],
                                start=(k == 0), stop=False,
                            )
                    nc.tensor.matmul(
                        out=pm[:, ns * 512 : (ns + 1) * 512],
                        lhsT=ones2[:], rhs=e2pair[:, col0 : col0 + 512],
                        start=False, stop=True,
                    )

            def mm_red(h, m):
                st_all = labf1 if h == 0 else labh1
                en_all = labj if h == 0 else labh
                pm = pm_tiles[(h, m)]
                # masked max over c != label (inverted single-index window)
                dmp = tmp_p.tile([P, 1], f32, tag="dmp", name=f"dmp_{h}_{m}")
                nc.vector._custom_dve(
                    TENSOR_MASK_REDUCE,
                    out=dmp[:].broadcast_to([P, HALF]),
                    in0=pm[:],
                    in1=en_all[:, m : m + 1],
                    s0=st_all[:, m : m + 1],
                    s1=NEG_BIG if h == 0 else acc0[:, m : m + 1],
                    imm2=1.0,
                    accum_out=(acc0 if h == 0 else negmax)[:, m : m + 1],
                )

            # label window coordinates: block-major layout [p, m] =
            # label[p*NT + m]; matrix column of class L is (L % CT)*P + L // CT
            # col = (label & 15) << 7 | (label >> 4), in exact int bit ops.
            # Emitted before any prep work (depends only on labi) so the
            # first masked scan can issue the moment its psum tile lands.
            lm = tmp_p.tile([P, NT], i32, tag="lm")
            nc.vector.tensor_scalar(out=lm[:], in0=labi[:], scalar1=15, scalar2=7,
                                    op0=Alu.bitwise_and, op1=Alu.logical_shift_left)
            ld = tmp_p.tile([P, NT], i32, tag="ld")
            nc.vector.tensor_scalar(out=ld[:], in0=labi[:], scalar1=4, scalar2=None,
                                    op0=Alu.logical_shift_right)
            nc.vector.tensor_tensor(out=lm[:], in0=lm[:], in1=ld[:], op=Alu.bitwise_or)
            labj = pp.tile([P, NT], f32)       # column index of label class
            nc.vector.tensor_copy(out=labj[:], in_=lm[:])
            labf1 = pp.tile([P, NT], f32)      # col + 1
            nc.vector.tensor_scalar_add(out=labf1[:], in0=labj[:], scalar1=1.0)
            labh = pp.tile([P, NT], f32)       # col - HALF
            nc.vector.tensor_scalar_add(out=labh[:], in0=labj[:], scalar1=float(-HALF))
            labh1 = pp.tile([P, NT], f32)      # col - HALF + 1
            nc.vector.tensor_scalar_add(out=labh1[:], in0=labj[:], scalar1=float(1 - HALF))

            # ---- emission schedule ----
            # Each engine's queue is in-order; the sequence below is laid out
            # so every queue reaches its critical ops before their consumers
            # need them: ACT alternates squares and eT evictions (e2/x2 for
            # groups 0-1 and the h0 eT columns land by the first GEMM), the
            # x2 squares of groups 2-3 run before their e2 squares (x2 gates
            # an -> tpa -> aT -> h0 GEMMs; e2 of groups 2-3 is h1-only), all
            # aT evictions live on the otherwise-idle GPSIMD, and the DVE
            # queue is scans + the few stat ops that must interleave.
            # x2 squares for groups 0-1 on ACT; e2 sums for groups 0-1 as
            # DVE fused multiply-reduces in DVE's startup idle window
            for t in range(4):
                sq_w(t)
            for t in range(8):
                ttr_e(t)
            for t in range(4, 8):
                sq_w(t)
            e2bits(0)
            e2bits(1)
            rnorm_an(0)
            rnorm_an(1)
            tpe(0)
            tpa(0)
            tpe(1)
            tpa(1)
            ev_e(0)
            ev_e(1)
            ev_a(0)
            ev_a(1)
            # groups 2-3: x2 first (gates an -> tpa -> aT -> h0 GEMMs),
            # e2 squares after (h1-only)
            for t in range(8, 16):
                sq_w(t)
            rnorm_an(2)
            rnorm_an(3)
            tpe(2)
            tpa(2)
            tpe(3)
            tpa(3)
            ev_e(2)
            ev_e(3)
            ev_a(2)
            ev_a(3)
            for t in range(8, 16):
                sq_e(t)
            # GEMM + masked-scan pipeline, scans one stage behind; the only
            # DVE insertions are the two late e2 bounce preps (h1-only)
            mm_mms(0, 0)
            mm_mms(0, 1)
            mm_red(0, 0)
            mm_mms(0, 2)
            mm_red(0, 1)
            mm_mms(0, 3)
            mm_red(0, 2)
            e2bits(2)
            mm_mms(0, 4)
            mm_red(0, 3)
            mm_mms(0, 5)
            mm_red(0, 4)
            mm_mms(0, 6)
            mm_red(0, 5)
            e2bits(3)
            mm_mms(0, 7)
            mm_red(0, 6)
            for m in range(8, 16):
                mm_mms(0, m)
                mm_red(0, m - 1)
            mm_red(0, 15)
            # second half of the gathers (their SWDGE queue slots only free
            # up once the first eight finish on the wire) and the
            # label-distance pipeline: emitted after the h0 pipeline so the
            # priority-driven scheduler keeps the Pool engine clear for the
            # GEMM-gating aT evictions first.
            for m in range(8, 16):
                gather(m)
            for m in range(16):
                label_dist(m)
            for m in range(8):
                mm_mms(1, m)
                mm_red(1, m)

            # label-side sqrt while the h1 scans are still draining:
            # ld2 is complete long before negmax (sqd ends ~20us earlier)
            ld2c = tmp_p.tile([P, NT], f32, tag="ld2c")
            nc.vector.tensor_scalar_max(out=ld2c[:], in0=ld2[:], scalar1=0.0)
            rl = _rsqrt(nc, tmp_p, ld2c[:], NT, "l")
            nc.vector.tensor_mul(out=rl[:], in0=rl[:], in1=ld2c[:])

            for m in range(8, 16):
                mm_mms(1, m)
                mm_red(1, m)

            # ---- epilogue ----
            # min_{c!=lab} d2 = 1 - negmax
            md2 = tmp_p.tile([P, NT], f32, tag="md2")
            nc.vector.tensor_scalar(out=md2[:], in0=negmax[:], scalar1=-1.0, scalar2=1.0,
                                    op0=Alu.mult, op1=Alu.add)
            nc.vector.tensor_scalar_max(out=md2[:], in0=md2[:], scalar1=0.0)
            # sqrt(x) = x * rsqrt(x); out = sqrt(ld2) - sqrt(md2)
            rm = _rsqrt(nc, tmp_p, md2[:], NT, "m")
            nc.vector.tensor_mul(out=rm[:], in0=rm[:], in1=md2[:])
            outv = pp.tile([P, NT], f32)
            nc.vector.tensor_sub(out=outv[:], in0=rl[:], in1=rm[:])
            nc.gpsimd.dma_start(out=out_d[:, :], in_=outv[:])

    nc.compile()
    return nc


_NC = None


def kernel(WO, emb_weight, label):
    global _NC
    if _NC is None:
        _NC = _build()

    WO = np.ascontiguousarray(np.asarray(WO, dtype=np.float32))
    emb = np.ascontiguousarray(np.asarray(emb_weight, dtype=np.float32))
    lab = np.asarray(label).astype(np.int32).reshape(N_FULL, 1)

    in_maps = []
    for i in range(N_CORES):
        sl = slice(i * NN, (i + 1) * NN)
        in_maps.append({
            "WO": WO[sl],
            "emb": emb,
            "label": np.ascontiguousarray(lab[sl]),
        })
    res = run_bass_kernel_spmd(_NC, in_maps, core_ids=list(range(N_CORES)))
    vals = np.stack([res.results[i]["out"] for i in range(N_CORES)])
    return np.float32(MARGIN + np.mean(vals.astype(np.float64)))
